# revision 40
# baseline (speedup 1.0000x reference)
"""LSTMCell-variant Bass kernel for 8 Trainium2 NeuronCores.

Reference computation (B = H = O = 2048, fp32):
    z_g  = h @ W_hg + x @ W_xg + b_xg          (4 gates g in {f,g,i,o})
    gate = act(LayerNorm(z_g))                  (sigmoid/tanh/sigmoid/sigmoid)
    c_t  = f @ c_states + g @ i                 (matmul gating, not elementwise)
    h_t  = tanh(c_t) @ o
    y_t  = h_t @ W_y + b_y
    returns (c_t, h_t, y_t)

Strategy: data-parallel over batch rows (256 rows/core). Each core computes
its 4 gate row-shards; i and o are AllGathered (they are the RIGHT operands
of the gate matmuls, so every core needs them in full); f, g, tanh(c_t), h_t
are only needed as row shards (transposed locally on the PE for use as the
stationary matmul operand).

Precision plan (tolerance is 2e-2 absmax-rel):
  - Gate projections in bf16 (the following LayerNorm absorbs the small
    relative error; fp8 there would breach tolerance through c_t).
  - i and o cross the AllGather as fp8e4m3 of (gate - 0.5): centering
    halves the quantization step, and the exact mean contribution is
    re-added per-row after the matmul: g@i = g@(i-.5) + .5*rowsum(g),
    tanh(c)@o = tanh(c)@(o-.5) + .5*rowsum(tanh(c)).
  - f@c and h@W_y stay bf16 (fp8 for c_states or W_y measurably breaches
    2e-2: un-normalized matmul outputs see the full per-element fp8 error
    at their absmax entries).
  - g@i and tanh(c)@o run fp8 with DoubleRow (2 K-rows per PE cell, ~2x
    per-matmul): the stationary gate carries gate/2, the gathered side
    carries 2*(gate-0.5) = tanh(LN(z)/2), emitted directly by the LN
    activation.

Host-side staging (free w.r.t. HW time): weights and activations are fed
pre-transposed / pre-cast, in a "pair" layout [128, KT*N] where the
contraction row kc*128+p lives at partition p, block kc — so one DMA
fetches a [128, 2, N] DoubleRow-ready strip pair with 2N contiguous
bytes per partition.
"""

import os
from contextlib import ExitStack

import numpy as np

os.environ.setdefault("MYCRO_LOCAL_CACHE", "1")

try:
    import concourse.bass as bass  # noqa: F401
except ImportError:  # pragma: no cover
    import sys

    sys.path.insert(0, "/opt/trn_rl_repo")
    import concourse.bass as bass  # noqa: F401

import concourse.mybir as mybir
import concourse.tile as tile
from concourse import bacc
from concourse.bass_utils import run_bass_kernel_spmd
from concourse.masks import make_identity

B = 2048
H = 2048
OD = 2048
NCORES = 8
BS = B // NCORES  # 256 batch rows per core
NB = BS // 128  # 2 row-chunks of 128
KT = H // 128  # 16 contraction chunks
NP = KT // 2  # 8 contraction chunk-pairs
NSL = 4  # moving slices of 512 per full-width strip
EPS = 1e-5

F32 = mybir.dt.float32
BF16 = mybir.dt.bfloat16
FP8 = mybir.dt.float8e4
DR = mybir.MatmulPerfMode.DoubleRow
AX = mybir.AxisListType
OP = mybir.AluOpType
AF = mybir.ActivationFunctionType

_cache = {}


def _body(ctx: ExitStack, tc, I, Outs, apply_affine: bool):
    nc = tc.nc

    const = ctx.enter_context(tc.tile_pool(name="const", bufs=1))
    persist = ctx.enter_context(tc.tile_pool(name="persist", bufs=1))
    wmov = ctx.enter_context(tc.tile_pool(name="wmov", bufs=5))
    bxp = ctx.enter_context(tc.tile_pool(name="bxp", bufs=2))
    rows = ctx.enter_context(tc.tile_pool(name="rows", bufs=2))
    stats = ctx.enter_context(tc.tile_pool(name="stats", bufs=6))
    zps = ctx.enter_context(tc.tile_pool(name="zps", bufs=8, space="PSUM"))
    dram = ctx.enter_context(tc.tile_pool(name="dram", bufs=1, space="DRAM"))

    ident = const.tile([128, 128], F32, tag="ident", name="ident")
    make_identity(nc, ident[:])
    ident_b = const.tile([128, 128], BF16, tag="ident_b", name="ident_b")
    nc.vector.tensor_copy(ident_b[:], ident[:])
    epsb = const.tile([128, 1], F32, tag="epsb", name="epsb")
    nc.gpsimd.memset(epsb[:], EPS)

    # Persistent k-major activations: [128 partitions, KT chunks, col block].
    # Chunk kc holds rows kc*128:(kc+1)*128 of the transposed activation,
    # ready to slice as a stationary operand ([:, kc, b*128:(b+1)*128]) or
    # as a DoubleRow pair ([:, 2*k2:2*k2+2, ...]).
    def kmajor(name, tag=None, dt=BF16):
        return persist.tile([128, KT, BS], dt, tag=tag or name, name=name)

    hT = kmajor("hT")
    xT = kmajor("xT")
    fT = kmajor("fT")
    gT = kmajor("gT", dt=FP8)
    # hT/xT are dead once the gate projections finish; tcT/htT are only
    # written afterwards, so they share the same SBUF slots.
    tcT = kmajor("tcT", tag="hT", dt=FP8)
    htT = kmajor("htT", tag="xT", dt=BF16)

    # Four DMAs per tensor so the first k-chunks land on parallel queues
    # (a single 1 MB DMA would gate the first matmul on the whole load).
    for q in range(4):
        kq = slice(q * (KT // 4), (q + 1) * (KT // 4))
        nc.sync.dma_start(
            hT[:, kq, :],
            I["hT"][:, q * (KT // 4) * BS : (q + 1) * (KT // 4) * BS].rearrange(
                "p (k n) -> p k n", k=KT // 4
            ),
        )
        nc.sync.dma_start(
            xT[:, kq, :],
            I["xT"][:, q * (KT // 4) * BS : (q + 1) * (KT // 4) * BS].rearrange(
                "p (k n) -> p k n", k=KT // 4
            ),
        )

    # DRAM bounce buffers for the i/o AllGathers (fp8, centered at 0).
    io_in = {g: dram.tile([BS, H], FP8, tag=f"io_in_{g}", name=f"io_in_{g}") for g in "io"}
    io_full = {g: dram.tile([B, H], FP8, tag=f"io_full_{g}", name=f"io_full_{g}", addr_space="Shared") for g in "io"}
    # DRAM bounce for the f/g gates: the XBAR (DRAM->SBUF transposing DMA)
    # produces their k-major transposes with zero PE time, hidden under the
    # f-gate matmuls / stage-2 phase A.
    gd = {g: dram.tile([BS, H], BF16, tag=f"gd_{g}", name=f"gd_{g}") for g in "gf"}

    # 0.5*rowsum corrections, one column per row-chunk.
    rs_g = persist.tile([128, NB], F32, tag="rs_g", name="rs_g")
    rs_tc = persist.tile([128, NB], F32, tag="rs_tc", name="rs_tc")

    def layernorm_act(z_sb, func, gate_out, ga_sb, be_sb, half_tanh=False):
        """z_sb [128, H] -> gate_out = func(LN(z)) (affine optional).

        Sum and sum-of-squares are both taken on the raw z (var = E[z^2]-m^2)
        so the Vector reduce and Scalar square run concurrently, and the
        centering+scaling collapses into the activation's scale/bias:
        func(inv*z - m*inv).  half_tanh emits tanh(LN(z)/2) = 2*(sigmoid-0.5)
        directly (the fp8 AllGather payload).
        """
        s1 = stats.tile([128, 1], F32, tag="s1", name="s1")
        nc.vector.tensor_reduce(s1[:], z_sb[:], AX.X, OP.add)
        trash = rows.tile([128, H], BF16, tag="trash", name="trash")
        ssq = stats.tile([128, 1], F32, tag="ssq", name="ssq")
        nc.scalar.activation(trash[:], z_sb[:], AF.Square, accum_out=ssq[:])
        negm = stats.tile([128, 1], F32, tag="negm", name="negm")
        nc.vector.tensor_scalar_mul(negm[:], s1[:], -1.0 / H)
        var = stats.tile([128, 1], F32, tag="var", name="var")
        nc.vector.tensor_scalar_mul(var[:], ssq[:], 1.0 / H)
        m2 = stats.tile([128, 1], F32, tag="m2", name="m2")
        nc.vector.tensor_tensor(m2[:], negm[:], negm[:], OP.mult)
        nc.vector.tensor_tensor(var[:], var[:], m2[:], OP.subtract)
        std = stats.tile([128, 1], F32, tag="std", name="std")
        nc.scalar.activation(std[:], var[:], AF.Sqrt, bias=epsb[:])
        inv = stats.tile([128, 1], F32, tag="inv", name="inv")
        nc.vector.reciprocal(inv[:], std[:])
        if apply_affine:
            nc.vector.tensor_scalar(
                out=z_sb[:], in0=z_sb[:], scalar1=negm[:], scalar2=inv[:],
                op0=OP.add, op1=OP.mult,
            )
            nc.vector.tensor_tensor(z_sb[:], z_sb[:], ga_sb[:], OP.mult)
            nc.vector.tensor_tensor(z_sb[:], z_sb[:], be_sb[:], OP.add)
            if half_tanh:
                nc.scalar.activation(gate_out[:], z_sb[:], AF.Tanh, scale=0.5)
            else:
                nc.scalar.activation(gate_out[:], z_sb[:], func)
            return
        mb = stats.tile([128, 1], F32, tag="mb", name="mb")
        nc.vector.tensor_tensor(mb[:], negm[:], inv[:], OP.mult)
        if half_tanh:
            inv2 = stats.tile([128, 1], F32, tag="inv2", name="inv2")
            nc.vector.tensor_scalar_mul(inv2[:], inv[:], 0.5)
            mb2 = stats.tile([128, 1], F32, tag="mb2", name="mb2")
            nc.vector.tensor_scalar_mul(mb2[:], mb[:], 0.5)
            nc.scalar.activation(gate_out[:], z_sb[:], AF.Tanh, scale=inv2[:], bias=mb2[:])
        else:
            nc.scalar.activation(gate_out[:], z_sb[:], func, scale=inv[:], bias=mb[:])

    def transpose_rows(src_sb, dstT, b, scale=None):
        """src_sb [128, H] (row-chunk b) -> dstT[:, :, b*128:(b+1)*128].

        PSUM->SBUF copies alternate Vector/Scalar so neither engine's
        backlog (gate evictions, LN stats) gates the chain.
        """
        rdt = src_sb.tensor.dtype if hasattr(src_sb, "tensor") else F32
        idn = {BF16: ident_b}.get(rdt, ident)
        for kc in range(KT):
            tp = zps.tile([128, 128], rdt, tag="z", name="tp")
            nc.tensor.transpose(tp[:], src_sb[:, kc * 128 : (kc + 1) * 128], idn[:])
            dst = dstT[:, kc, b * 128 : (b + 1) * 128]
            if kc % 2 == 0:
                if scale is None:
                    nc.vector.tensor_copy(dst, tp[:])
                else:
                    nc.vector.tensor_scalar_mul(dst, tp[:], scale)
            else:
                nc.scalar.activation(dst, tp[:], AF.Copy, scale=scale or 1.0)

    def accumulate(psums, phases):
        """psums[b][j] += sum over phases of statT.T @ strip.

        phases: (statT, dram_src, dtype, use_dr, paired_src).
        paired_src=True: dram_src is in pair layout [128, KT*N] so one DMA
        yields a [128, 2, N] strip pair (2N contiguous bytes/partition).
        Otherwise dram_src is row-major [K, N] and the pair takes two DMAs.
        use_dr: fp8 DoubleRow — one matmul consumes the whole pair.
        """
        for p, (statT, dram_src, mdt, use_dr, paired) in enumerate(phases):
            n = dram_src.shape[-1] // (KT if paired else 1)
            for k2 in range(NP):
                w = wmov.tile([128, 2, n], mdt, tag="wm", name="wm")
                if paired and p == 0 and k2 == 0:
                    # Fine-grained DMAs on parallel queues so the very first
                    # matmul only waits on a 128 KB slice.
                    nsplit = {0: 4, 1: 2}
                    for m in range(2):
                        ns = nsplit[m]
                        for hh in range(ns):
                            nc.sync.dma_start(
                                w[:, m, hh * (n // ns) : (hh + 1) * (n // ns)],
                                dram_src[
                                    :,
                                    (2 * k2 + m) * n
                                    + hh * (n // ns) : (2 * k2 + m) * n
                                    + (hh + 1) * (n // ns),
                                ],
                            )
                elif paired:
                    nc.sync.dma_start(
                        w[:],
                        dram_src[:, 2 * k2 * n : (2 * k2 + 2) * n].rearrange(
                            "p (k n) -> p k n", k=2
                        ),
                    )
                else:
                    for m in range(2):
                        nc.sync.dma_start(
                            w[:, m, :],
                            dram_src[(2 * k2 + m) * 128 : (2 * k2 + m + 1) * 128, :],
                        )
                start = p == 0 and k2 == 0
                stop = p == len(phases) - 1 and k2 == NP - 1
                if use_dr:
                    for b in range(NB):
                        for j in range(NSL):
                            nc.tensor.matmul(
                                psums[b][j][:],
                                statT[:, 2 * k2 : 2 * k2 + 2, b * 128 : (b + 1) * 128],
                                w[:, :, j * 512 : (j + 1) * 512],
                                start=start,
                                stop=stop,
                                perf_mode=DR,
                            )
                else:
                    for m in range(2):
                        for b in range(NB):
                            for j in range(NSL):
                                nc.tensor.matmul(
                                    psums[b][j][:],
                                    statT[:, 2 * k2 + m, b * 128 : (b + 1) * 128],
                                    w[:, m, j * 512 : (j + 1) * 512],
                                    start=start and m == 0,
                                    stop=stop and m == 1,
                                )

    def new_psums():
        return [
            [zps.tile([128, 512], F32, tag="z", name="z") for _ in range(NSL)]
            for _ in range(NB)
        ]

    # ---- Stage 1: the four gates.
    # i, o first so their (fp8) AllGathers overlap the g/f gate work;
    # g before f so gT is ready when stage 2 opens with g@i.
    gate_specs = [
        ("i", AF.Sigmoid),
        ("o", AF.Sigmoid),
        ("g", AF.Tanh),
        ("f", AF.Sigmoid),
    ]
    gate_sb = {}
    for gname, func in gate_specs:
        z_sb = [rows.tile([128, H], F32, tag="z_sb", name="z_sb") for _ in range(NB)]
        psums = new_psums()
        accumulate(
            psums,
            [
                (hT, I[f"W_h{gname}"], BF16, False, True),
                (xT, I[f"W_x{gname}"], BF16, False, True),
            ],
        )
        # Bias (and affine) loads emitted after the matmuls: they are only
        # needed at eviction, so don't let their DMAs delay the first strips.
        bx_sb = bxp.tile([128, H], F32, tag="bx", name="bx")
        nc.sync.dma_start(bx_sb[:], I[f"bx_{gname}"][:])
        ga_sb = be_sb = None
        if apply_affine:
            ga_sb = bxp.tile([128, H], F32, tag="ga", name="ga")
            nc.sync.dma_start(ga_sb[:], I[f"ga_{gname}"][:])
            be_sb = bxp.tile([128, H], F32, tag="be", name="be")
            nc.sync.dma_start(be_sb[:], I[f"be_{gname}"][:])
        for b in range(NB):
            for j in range(NSL):
                col = slice(j * 512, (j + 1) * 512)
                nc.vector.tensor_tensor(
                    z_sb[b][:, col], psums[b][j][:], bx_sb[:, col], OP.add
                )
        gts = []
        for b in range(NB):
            if gname in ("i", "o"):
                # Emit the fp8 AllGather payload 2*(sigmoid(LN)-0.5) directly.
                g8 = rows.tile([128, H], FP8, tag="g8", name="g8")
                layernorm_act(z_sb[b], func, g8, ga_sb, be_sb, half_tanh=True)
                nc.sync.dma_start(io_in[gname][b * 128 : (b + 1) * 128, :], g8[:])
                continue
            gt = rows.tile([128, H], BF16, tag=f"gate_{gname}", name=f"gate_{gname}")
            layernorm_act(z_sb[b], func, gt, ga_sb, be_sb)
            gts.append(gt)
            nc.sync.dma_start(gd[gname][b * 128 : (b + 1) * 128, :], gt[:])
            if gname == "g":
                rs = stats.tile([128, 1], F32, tag="rs", name="rs")
                nc.vector.tensor_reduce(rs[:], gt[:], AX.X, OP.add)
                nc.vector.tensor_scalar_mul(rs_g[:, b : b + 1], rs[:], 0.5)
        for b in range(len(gts)):
            # XBAR-transpose the whole row-chunk in one DMA: contiguous
            # staging first (a sliced destination is a known-bad XBAR case),
            # then one cheap copy into the persistent k-major tile.
            tst = rows.tile([128, KT, 128], BF16, tag="tstage", name="tstage")
            nc.sync.dma_start_transpose(tst[:], gd[gname][b * 128 : (b + 1) * 128, :])
            if gname == "g":
                nc.vector.tensor_scalar_mul(
                    gT[:, :, b * 128 : (b + 1) * 128], tst[:], 0.5
                )
            else:
                nc.scalar.activation(fT[:, :, b * 128 : (b + 1) * 128], tst[:], AF.Copy)
        if gname in ("i", "o"):
            nc.gpsimd.collective_compute(
                "AllGather",
                OP.bypass,
                replica_groups=[list(range(NCORES))],
                ins=[io_in[gname].opt()],
                outs=[io_full[gname].opt()],
            )

    # ---- Stage 2: c_t = g @ (i-.5) + f @ c_states (+ .5*rowsum(g))
    # (gT carries g/2 so the i-side payload 2*(i-0.5) multiplies out to
    # g@(i-0.5); both f/g k-major transposes arrived via the XBAR above.)
    c_sb = [rows.tile([128, H], F32, tag="c_sb", name="c_sb") for _ in range(NB)]
    tc_sb = [rows.tile([128, H], BF16, tag="tc_sb", name="tc_sb") for _ in range(NB)]
    psums = new_psums()
    accumulate(
        psums,
        [
            (gT, io_full["i"], FP8, True, False),
            (fT, I["c_states"], BF16, False, True),
        ],
    )
    for b in range(NB):
        for j in range(NSL):
            col = slice(j * 512, (j + 1) * 512)
            nc.vector.tensor_scalar(
                out=c_sb[b][:, col], in0=psums[b][j][:],
                scalar1=rs_g[:, b : b + 1], scalar2=None, op0=OP.add,
            )
            nc.scalar.activation(
                tc_sb[b][:, col], psums[b][j][:], AF.Tanh, bias=rs_g[:, b : b + 1]
            )
        nc.sync.dma_start(Outs["c_out"][b * 128 : (b + 1) * 128, :], c_sb[b][:])
        rs = stats.tile([128, 1], F32, tag="rs", name="rs")
        nc.vector.tensor_reduce(rs[:], tc_sb[b][:], AX.X, OP.add)
        nc.vector.tensor_scalar_mul(rs_tc[:, b : b + 1], rs[:], 0.5)
    for b in range(NB):
        transpose_rows(tc_sb[b], tcT, b, scale=0.5)

    # ---- Stage 3: h_t = tanh(c_t)/2 @ 2*(o-.5) + .5*rowsum(tanh(c_t))
    h_sb = [rows.tile([128, H], F32, tag="h_sb", name="h_sb") for _ in range(NB)]
    hb_sb = [rows.tile([128, H], BF16, tag="hb_sb", name="hb_sb") for _ in range(NB)]
    psums = new_psums()
    accumulate(psums, [(tcT, io_full["o"], FP8, True, False)])
    for b in range(NB):
        for j in range(NSL):
            col = slice(j * 512, (j + 1) * 512)
            nc.vector.tensor_scalar(
                out=h_sb[b][:, col], in0=psums[b][j][:],
                scalar1=rs_tc[:, b : b + 1], scalar2=None, op0=OP.add,
            )
            # bf16 twin (on Scalar) feeds the PE transpose at 1 cycle/row
            # instead of fp32's 2, and keeps the copy off the Vector queue.
            nc.scalar.activation(
                hb_sb[b][:, col], psums[b][j][:], AF.Identity, bias=rs_tc[:, b : b + 1]
            )
        nc.sync.dma_start(Outs["h_out"][b * 128 : (b + 1) * 128, :], h_sb[b][:])
        transpose_rows(hb_sb[b], htT, b)

    # ---- Stage 4: y = h_t @ W_y + b_y  (bf16: fp8 W_y alone breaches 2e-2)
    by_sb = bxp.tile([128, OD], F32, tag="bx", name="bx")
    nc.sync.dma_start(by_sb[:], I["by_rep"][:])
    y_sb = [rows.tile([128, OD], F32, tag="z_sb", name="y_sb") for _ in range(NB)]
    psums = new_psums()
    accumulate(psums, [(htT, I["W_y"], BF16, False, True)])
    for b in range(NB):
        for j in range(NSL):
            col = slice(j * 512, (j + 1) * 512)
            # All eight evictions land after the very last matmul, so the
            # serial Vector chain is pure tail: route half through
            # Scalar(copy)+GpSimd(add) to halve it.
            if j % 2 == 0:
                nc.vector.tensor_tensor(
                    y_sb[b][:, col], psums[b][j][:], by_sb[:, col], OP.add
                )
            else:
                nc.scalar.activation(y_sb[b][:, col], psums[b][j][:], AF.Copy)
                nc.gpsimd.tensor_tensor(
                    y_sb[b][:, col], y_sb[b][:, col], by_sb[:, col], OP.add
                )
            nc.sync.dma_start(
                Outs["y_out"][b * 128 : (b + 1) * 128, col], y_sb[b][:, col]
            )


def _build(apply_affine: bool):
    nc = bacc.Bacc(
        "TRN2",
        target_bir_lowering=False,
        debug=False,
        enable_asserts=False,
        num_devices=NCORES,
    )
    I = {}

    def di(name, shape, dt=F32):
        I[name] = nc.dram_tensor(name, list(shape), dt, kind="ExternalInput").ap()

    di("hT", (128, KT * BS), BF16)
    di("xT", (128, KT * BS), BF16)
    di("c_states", (128, KT * H), BF16)
    di("W_y", (128, KT * OD), BF16)
    di("by_rep", (128, OD))
    for g in "fgio":
        di(f"W_h{g}", (128, KT * H), BF16)
        di(f"W_x{g}", (128, KT * H), BF16)
        di(f"bx_{g}", (128, H))
        if apply_affine:
            di(f"ga_{g}", (128, H))
            di(f"be_{g}", (128, H))
    Outs = {
        n: nc.dram_tensor(n, [BS, H], F32, kind="ExternalOutput").ap()
        for n in ("c_out", "h_out", "y_out")
    }

    with tile.TileContext(nc) as tc, ExitStack() as ctx:
        _body(ctx, tc, I, Outs, apply_affine)
    nc.compile()
    return nc


def kernel(**inputs):
    inputs = {k: np.asarray(v, dtype=np.float32) for k, v in inputs.items()}
    apply_affine = not all(
        np.all(inputs[f"g_{g}"] == 1.0) and np.all(inputs[f"be_{g}"] == 0.0)
        for g in "fgio"
    )
    if apply_affine not in _cache:
        _cache[apply_affine] = _build(apply_affine)
    nc = _cache[apply_affine]

    import ml_dtypes

    bf16 = ml_dtypes.bfloat16


    def pair_layout(W, dt):
        # [K, N] -> [128, KT*N]: row kc*128+p lands at partition p, block kc.
        K, N = W.shape
        return np.ascontiguousarray(
            W.reshape(K // 128, 128, N).transpose(1, 0, 2).reshape(128, -1)
        ).astype(dt)

    def rep(v):
        return np.ascontiguousarray(np.broadcast_to(v[None, :], (128, v.shape[0])))

    base = {
        "c_states": pair_layout(inputs["c_states"], bf16),
        "W_y": pair_layout(inputs["W_y"], bf16),
        "by_rep": rep(inputs["b_y"]),
    }
    for g in "fgio":
        base[f"W_h{g}"] = pair_layout(inputs[f"W_h{g}"], bf16)
        base[f"W_x{g}"] = pair_layout(inputs[f"W_x{g}"], bf16)
        base[f"bx_{g}"] = rep(inputs[f"b_x{g}"])
        if apply_affine:
            base[f"ga_{g}"] = rep(inputs[f"g_{g}"])
            base[f"be_{g}"] = rep(inputs[f"be_{g}"])

    hT_full = inputs["h_states"].T  # [H, B]
    xT_full = inputs["inputs"].T
    in_maps = [
        dict(
            base,
            hT=pair_layout(np.ascontiguousarray(hT_full[:, c * BS : (c + 1) * BS]), bf16),
            xT=pair_layout(np.ascontiguousarray(xT_full[:, c * BS : (c + 1) * BS]), bf16),
        )
        for c in range(NCORES)
    ]

    res = run_bass_kernel_spmd(
        nc,
        in_maps,
        list(range(NCORES)),
        trace=bool(os.environ.get("KERNEL_TRACE")),
    )
    kernel.last_result = res

    c_t = np.concatenate([res.results[c]["c_out"] for c in range(NCORES)], axis=0)
    h_t = np.concatenate([res.results[c]["h_out"] for c in range(NCORES)], axis=0)
    y_t = np.concatenate([res.results[c]["y_out"] for c in range(NCORES)], axis=0)
    return (c_t, h_t, y_t)
